# revision 15
# baseline (speedup 1.0000x reference)
"""Trainium2 kernel for nn_ContConv1dDense (banded continuous conv with
kernel-MLP), data-parallel over (batch, sequence-half) on 8 NeuronCores.

Math: the reference computes, per (b, i, k in 1..8):
    dt      = (times[b,i] - times[b,i-k]) masked to the band & valid length
    hidden  = relu(dt * W1 + b1)                       # (128,)
    kv      = (hidden @ W2 + b2).reshape(32, 32)       # masked
    out[b,i,:] += features[b,i-k,:] @ kv

For this operator's input family, `times` is sorted along the sequence axis
(so dt >= 0) and b1 == b2 == 0.  Then relu(dt*W1) == dt * max(W1, 0)
exactly, and the whole kernel-MLP collapses to a *constant* 32x32 matrix
V = (max(W1,0) @ W2).reshape(32,32).  Reassociating the contraction:

    out[b,i,:] = (sum_k dt_m[b,i,k] * features[b,i-k,:]) @ V = g[b,i,:] @ V

This is an exact algebraic identity for those inputs (verified by the guard
below at runtime; a general fallback handles anything else).

Per-core device program (core = 2*b + half, 1024 positions each):
  1. Feature windows fwin[p,t,q,:] = ft[128t+p+q, :] gathered straight from
     the padded f16 feature input in DRAM -- no staging, no dependencies, all
     8 tile gathers issue immediately across 4 queues.
  2. dt tiles [128 pos, 8 k] from shifted window loads of `times`, masked by
     a single host-precomputed band&length mask, cast to f16.
  3. g = sum_k dt*fwin via f16 broadcast-multiply + X-axis reduce on the DVE.
  4. Tail on the (otherwise idle) PE: transpose g via identity matmul
     ([128,128] per 4 tiles), then per-tile gT @ V in bf16; ACT copies
     PSUM->SBUF; output DMAs spread over the Sync and GpSimd queues.
"""

import os

import numpy as np

_STAGE = int(os.environ.get("KSTAGE", "0"))

KS = 8          # band width (kernel size)
B = 4
L = 2048
C = 32          # in channels
OUT = 32        # out channels
HALF = 1024     # positions per core
PAD = 8         # halo rows in front of each shard
SEQ = HALF + PAD
NT = HALF // 128  # 8 position-tiles per core
N_CORES = 8

_CACHE = {}


def _build_program_v2():
    from contextlib import ExitStack

    import concourse.bacc as bacc
    import concourse.bass as bass
    from concourse import mybir

    f32 = mybir.dt.float32
    f16 = mybir.dt.float16
    bf16 = mybir.dt.bfloat16

    nc = bacc.Bacc(
        "TRN2", target_bir_lowering=False, debug=False, num_devices=N_CORES
    )

    tm = nc.dram_tensor("tm", [SEQ], f32, kind="ExternalInput").ap()
    mk = nc.dram_tensor("mk", [128, NT * KS], f32, kind="ExternalInput").ap()
    ft = nc.dram_tensor("ft", [SEQ, C], f16, kind="ExternalInput").ap()
    vm = nc.dram_tensor("vm", [128, OUT], bf16, kind="ExternalInput").ap()
    idm = nc.dram_tensor("idm", [128, 128], f32, kind="ExternalInput").ap()
    out = nc.dram_tensor("out", [HALF, OUT], f32, kind="ExternalOutput").ap()

    ta = nc.alloc_sbuf_tensor("ta", [128, NT, KS + 1], f32).ap()
    mk_sb = nc.alloc_sbuf_tensor("mk_sb", [128, NT, KS], f32).ap()
    dtr = nc.alloc_sbuf_tensor("dtr", [128, NT, KS], f32).ap()
    dth = nc.alloc_sbuf_tensor("dth", [128, NT, KS], f16).ap()
    fwin = nc.alloc_sbuf_tensor("fwin", [128, NT, KS, C], f16).ap()
    # product [p, t, q, c] fully contiguous; summed over q by tree adds
    pr = nc.alloc_sbuf_tensor("pr", [128, NT, KS, C], f16).ap()
    s1 = nc.alloc_sbuf_tensor("s1", [128, NT, KS // 2, C], f16).ap()
    s2 = nc.alloc_sbuf_tensor("s2", [128, NT, KS // 4, C], f16).ap()
    oh = nc.alloc_sbuf_tensor("oh", [128, NT, C], f32).ap()
    gtc = [nc.alloc_sbuf_tensor(f"gtc{i}", [64, 128], bf16).ap() for i in range(4)]
    osb = nc.alloc_sbuf_tensor("osb", [128, NT, OUT], f32).ap()
    id_sb = nc.alloc_sbuf_tensor("id_sb", [128, 128], f32).ap()
    vm_sb = nc.alloc_sbuf_tensor("vm_sb", [128, OUT], bf16).ap()
    scr = nc.alloc_sbuf_tensor("scr", [1, 1], f32).ap()

    # one full PSUM bank per buffer so PE writes and ACT reads of
    # back-to-back stages never touch the same bank
    psT = [nc.alloc_psum_tensor(f"psT{i}", [128, 512], f32).ap() for i in range(2)]
    po = [nc.alloc_psum_tensor(f"po{i}", [128, 512], f32).ap() for i in range(4)]

    with ExitStack() as _sctx:
        block = _sctx.enter_context(nc.Block(no_gpsimd_drain=True))
        _names = ["sIN", "sMK", "sGA", "sGAg", "sGB", "sGBg", "sID",
                  "sVM", "sVD", "sGD", "sOH", "sOHg", "sPE", "sCP", "sOS",
                  "sOUT", "sOUTg"]
        _sems = {n: _sctx.enter_context(nc.semaphore(n)) for n in _names}
        (sIN, sMK, sGA, sGAg, sGB, sGBg, sID, sVM, sVD, sGD, sOH, sOHg,
         sPE, sCP, sOS, sOUT, sOUTg) = (_sems[n] for n in _names)

        def gather(raw, t, sem):
            # fwin[p, t, q, :] = ft[128t + p + q, :]; rows overlap, each
            # partition reads 8 contiguous 32-ch rows (512B) from DRAM.
            raw.dma_start(
                fwin[:, t, :, :],
                bass.AP(tensor=ft.tensor, offset=128 * t * C,
                        ap=[[C, 128], [C, KS], [1, C]]),
            ).then_inc(sem, 16)

        def slot(t):
            # 4 distinct PSUM out banks, matmul dst at bank col 0
            return po[t % 4][:, 0:OUT]

        def out_dma(raw, t, sem):
            raw.wait_ge(sOS, t + 1)
            raw.dma_start(
                bass.AP(tensor=out.tensor, offset=t * 128 * OUT,
                        ap=[[OUT, 128], [1, OUT]]),
                osb[:, t, :],
            ).then_inc(sem, 16)

        @block.sync
        def _(sync):
            sync.dma_start(
                ta[:],
                bass.AP(tensor=tm.tensor, offset=0,
                        ap=[[1, 128], [128, NT], [1, KS + 1]]),
            ).then_inc(sIN, 16)
            gather(sync, 0, sGA)
            gather(sync, 5, sGB)
            for t in range(4):
                out_dma(sync, t, sOUT)
            sync.wait_ge(sOUT, 96)
            sync.wait_ge(sOUTg, 32)

        @block.gpsimd
        def _(g):
            g.dma_start(mk_sb[:], mk[:]).then_inc(sMK, 16)
            gather(g, 1, sGAg)
            gather(g, 4, sGBg)
            g.dma_start(id_sb[:], idm[:]).then_inc(sID, 16)
            g.dma_start(vm_sb[:], vm[:]).then_inc(sVM, 16)
            # band contraction for tiles 6-7 runs here, overlapping the DVE
            g.wait_ge(sVD, 2)    # dth ready
            g.wait_ge(sGB, 48)
            g.wait_ge(sGBg, 16)
            sl = slice(6, 8)
            nc.gpsimd.tensor_tensor(
                pr[:, sl], fwin[:, sl],
                dth[:, sl, :, None].to_broadcast([128, 2, KS, C]),
                mybir.AluOpType.mult,
            ).then_inc(sGD, 1)
            g.wait_ge(sGD, 1)
            nc.gpsimd.tensor_tensor(
                s1[:, sl], pr[:, sl, 0:4, :], pr[:, sl, 4:8, :],
                mybir.AluOpType.add,
            ).then_inc(sGD, 1)
            g.wait_ge(sGD, 2)
            nc.gpsimd.tensor_tensor(
                s2[:, sl], s1[:, sl, 0:2, :], s1[:, sl, 2:4, :],
                mybir.AluOpType.add,
            ).then_inc(sGD, 1)
            g.wait_ge(sGD, 3)
            nc.gpsimd.tensor_tensor(
                oh[:, sl], s2[:, sl, 0, :], s2[:, sl, 1, :],
                mybir.AluOpType.add,
            ).then_inc(sOHg, 1)
            for t in (4, 5):
                out_dma(g, t, sOUTg)

        @block.scalar
        def _(s):
            gather(s, 2, sGA)
            gather(s, 3, sGA)
            gather(s, 6, sGB)
            gather(s, 7, sGB)
            # dummy activate: pulls the ACT table load off the critical path
            # (first ACTIVATE triggers a ~1.3us table fetch); osb[0,0,0] is
            # rewritten in-order by the real copy below.
            s.wait_ge(sMK, 16)
            nc.scalar.copy(scr[:], mk_sb[0:1, 0, 0:1])
            if _STAGE == 1:
                # debug: bypass PE tail, copy oh straight out (wrong values)
                for t in range(8):
                    s.wait_ge(sOH, 1 if t < 4 else 2)
                    nc.scalar.copy(osb[:, t, :], oh[:, t, :]).then_inc(sOS, 1)
            elif _STAGE == 2:
                # debug: transposes only; copy psT chunks out (wrong values)
                for g in range(4):
                    s.wait_ge(sPE, g + 1)
                    nc.scalar.copy(
                        osb[0:64, 2 * g:2 * g + 2, :], psT[g % 2][0:64, 0:64]
                    ).then_inc(sOS, 2)
            else:
                # (gtc chunk ready at sPE, src bank) then osb copies per MM
                s.wait_ge(sPE, 1)
                nc.scalar.copy(gtc[0][:], psT[0][0:64, 0:128]).then_inc(sCP, 1)
                s.wait_ge(sPE, 2)
                nc.scalar.copy(gtc[1][:], psT[1][0:64, 0:128]).then_inc(sCP, 1)
                for t in range(4):
                    s.wait_ge(sPE, t + 3)
                    nc.scalar.copy(osb[:, t, :], slot(t)).then_inc(sOS, 1)
                s.wait_ge(sPE, 7)
                nc.scalar.copy(gtc[2][:], psT[0][0:64, 0:128]).then_inc(sCP, 1)
                s.wait_ge(sPE, 8)
                nc.scalar.copy(osb[:, 4, :], slot(4)).then_inc(sOS, 1)
                s.wait_ge(sPE, 9)
                nc.scalar.copy(osb[:, 5, :], slot(5)).then_inc(sOS, 1)
                s.wait_ge(sPE, 10)
                nc.scalar.copy(gtc[3][:], psT[1][0:64, 0:128]).then_inc(sCP, 1)
                s.wait_ge(sPE, 11)
                nc.scalar.copy(osb[:, 6, :], slot(6)).then_inc(sOS, 1)
                s.wait_ge(sPE, 12)
                nc.scalar.copy(osb[:, 7, :], slot(7)).then_inc(sOS, 1)
                out_dma(s, 6, sOUT)
                out_dma(s, 7, sOUT)

        @block.tensor
        def _(te):
            if _STAGE == 1:
                return
            if _STAGE == 2:
                te.wait_ge(sID, 16)
                for g in range(4):
                    te.wait_ge(sOH, 1 if g < 2 else 2)
                    if g >= 2:
                        te.wait_ge(sOS, 2 * g)  # psT[g%2] drained
                    nc.tensor.transpose(
                        psT[g % 2][0:64, 0:128],
                        oh[:, 2 * g:2 * g + 2, :]
                        .rearrange("p a b -> p (a b)"),
                        id_sb[:],
                    ).then_inc(sPE, 1)
                return
            # transposes per 2 tiles ([64,128] chunks; operand base
            # partitions are restricted to {0, 32, 64}), then gT @ V matmuls
            def trans(pair, bank):
                nc.tensor.transpose(
                    psT[bank][0:64, 0:128],
                    oh[:, 2 * pair:2 * pair + 2, :]
                    .rearrange("p a b -> p (a b)"),
                    id_sb[:],
                ).then_inc(sPE, 1)

            def mm(t):
                te.wait_ge(sCP, t // 2 + 1)
                if t >= 4:
                    te.wait_ge(sOS, t - 3)  # slot(t-4) drained
                pb = 32 * (t % 2)
                nc.tensor.matmul(
                    slot(t), gtc[t // 2][pb:pb + 32, :],
                    vm_sb[pb:pb + 32, :], start=True, stop=True,
                ).then_inc(sPE, 1)

            te.wait_ge(sOH, 1)
            te.wait_ge(sID, 16)
            trans(0, 0)
            trans(1, 1)
            te.wait_ge(sVM, 16)
            for t in (0, 1, 2, 3):
                mm(t)
            te.wait_ge(sOH, 2)
            trans(2, 0)  # psT[0] drained by gtc0 copy (sCP>=1 via mm waits)
            mm(4)
            mm(5)
            te.wait_ge(sOHg, 1)
            trans(3, 1)
            mm(6)
            mm(7)

        @block.vector
        def _(v):
            v.wait_ge(sIN, 16)
            v.wait_ge(sMK, 16)
            nc.vector.tensor_tensor(
                dtr[:],
                ta[:, :, KS:KS + 1].to_broadcast([128, NT, KS]),
                ta[:, :, 0:KS],
                mybir.AluOpType.subtract,
            ).then_inc(sVD, 1)
            v.wait_ge(sVD, 1)
            nc.vector.tensor_tensor(
                dth[:], dtr[:], mk_sb[:], mybir.AluOpType.mult
            ).then_inc(sVD, 1)
            v.wait_ge(sVD, 2)
            nvd = [2]
            def chained(ins):
                nvd[0] += 1
                ins.then_inc(sVD, 1)
                v.wait_ge(sVD, nvd[0])
            for gi, (t0, nt) in enumerate(((0, 4), (4, 2))):
                if gi == 0:
                    v.wait_ge(sGA, 48)
                    v.wait_ge(sGAg, 16)
                elif gi == 1:
                    v.wait_ge(sGB, 48)
                    v.wait_ge(sGBg, 16)
                sl = slice(t0, t0 + nt)
                chained(nc.vector.tensor_tensor(
                    pr[:, sl], fwin[:, sl],
                    dth[:, sl, :, None].to_broadcast([128, nt, KS, C]),
                    mybir.AluOpType.mult,
                ))
                chained(nc.vector.tensor_tensor(
                    s1[:, sl], pr[:, sl, 0:4, :], pr[:, sl, 4:8, :],
                    mybir.AluOpType.add,
                ))
                chained(nc.vector.tensor_tensor(
                    s2[:, sl], s1[:, sl, 0:2, :], s1[:, sl, 2:4, :],
                    mybir.AluOpType.add,
                ))
                nc.vector.tensor_tensor(
                    oh[:, sl], s2[:, sl, 0, :], s2[:, sl, 1, :],
                    mybir.AluOpType.add,
                ).then_inc(sOH, 1)

    nc.compile()
    return nc


def _get_program():
    if "nc" not in _CACHE:
        _CACHE["nc"] = _build_program_v2()
    return _CACHE["nc"]


def _fast_path_ok(times, b1, b2):
    # The linearization relu(dt*W1 + b1) == dt*max(W1,0) is exact iff
    # b1 == 0 and dt >= 0 (times sorted); b2 == 0 removes the bias term.
    if np.any(b1 != 0.0) or np.any(b2 != 0.0):
        return False
    if np.any(np.diff(times, axis=1) < 0.0):
        return False
    return True


def _reference_fallback(times, features, lengths, W1, b1, W2, b2):
    # Straight numpy transcription of the reference (general inputs).
    Bn, Ln = times.shape
    offsets = np.arange(1, KS + 1)
    idx = np.arange(Ln)[:, None] - offsets[None, :]
    in_band = idx >= 0
    idx_c = np.clip(idx, 0, Ln - 1)
    t_j = times[:, idx_c]
    dt = times[:, :, None] - t_j
    pos_i = np.arange(Ln)[None, :, None]
    mask = (
        in_band[None]
        & (idx_c[None] < lengths[:, None, None])
        & (pos_i <= lengths[:, None, None] - 1)
    )
    dt = np.where(mask, dt, 0.0).astype(np.float32)
    hidden = np.maximum(dt[..., None] * W1[0] + b1, 0.0)
    kv = (hidden @ W2 + b2).reshape(Bn, Ln, KS, C, OUT)
    kv = np.where(mask[..., None, None], kv, 0.0)
    feat_g = features[:, idx_c]
    return np.einsum("blkc,blkco->blo", feat_g, kv).astype(np.float32)


def _build_in_maps(times, features, lengths, W1, W2):
    from ml_dtypes import bfloat16

    # Fold the (now linear) kernel-MLP into one 32x32 matrix.
    v = (np.maximum(W1[0], 0.0) @ W2).reshape(C, OUT).astype(np.float32)
    vm16 = np.ascontiguousarray(np.tile(v, (4, 1))).astype(bfloat16)
    ident = np.eye(128, dtype=np.float32)

    p_loc = np.arange(128)[:, None, None] + 128 * np.arange(NT)[None, :, None]
    k = KS - np.arange(KS)[None, None, :]

    in_maps = []
    for core in range(N_CORES):
        b, half = core // 2, core % 2
        start = half * HALF
        ftp = np.zeros((SEQ, C), np.float16)
        tmv = np.empty((SEQ,), np.float32)
        lo = start - PAD
        if lo < 0:
            ftp[PAD:] = features[b, 0:start + HALF]
            tmv[:PAD] = times[b, 0]
            tmv[PAD:] = times[b, 0:start + HALF]
        else:
            ftp[:] = features[b, lo:start + HALF]
            tmv[:] = times[b, lo:start + HALF]
        # mask[p, t, q] = 1 iff global pos i = start+128t+p has i >= 8-q
        # (band: j = i-k >= 0, k = 8-q) and local pos < lengths[b]-start.
        band = (p_loc + start) >= k
        lenm = p_loc < (int(lengths[b]) - start)
        mkv = np.ascontiguousarray(
            (band & lenm).astype(np.float32).reshape(128, NT * KS)
        )
        in_maps.append({"tm": tmv, "mk": mkv, "ft": ftp, "vm": vm16,
                        "idm": ident})
    return in_maps


def kernel(times, features, lengths, W1, b1, W2, b2):
    times = np.asarray(times, dtype=np.float32)
    features = np.asarray(features, dtype=np.float32)
    lengths = np.asarray(lengths)
    W1 = np.asarray(W1, dtype=np.float32)
    b1 = np.asarray(b1, dtype=np.float32)
    W2 = np.asarray(W2, dtype=np.float32)
    b2 = np.asarray(b2, dtype=np.float32)

    if not _fast_path_ok(times, b1, b2):
        return _reference_fallback(times, features, lengths, W1, b1, W2, b2)

    from concourse.bass_utils import run_bass_kernel_spmd

    nc = _get_program()
    in_maps = _build_in_maps(times, features, lengths, W1, W2)
    res = run_bass_kernel_spmd(nc, in_maps, core_ids=list(range(N_CORES)))

    out = np.empty((B, L, OUT), np.float32)
    for core in range(N_CORES):
        b, half = core // 2, core % 2
        out[b, half * HALF : (half + 1) * HALF, :] = res.results[core]["out"]
    return out


# revision 16
# speedup vs baseline: 1.0412x; 1.0412x over previous
"""Trainium2 kernel for nn_ContConv1dDense (banded continuous conv with
kernel-MLP), data-parallel over (batch, sequence-half) on 8 NeuronCores.

Math: the reference computes, per (b, i, k in 1..8):
    dt      = (times[b,i] - times[b,i-k]) masked to the band & valid length
    hidden  = relu(dt * W1 + b1)                       # (128,)
    kv      = (hidden @ W2 + b2).reshape(32, 32)       # masked
    out[b,i,:] += features[b,i-k,:] @ kv

For this operator's input family, `times` is sorted along the sequence axis
(so dt >= 0) and b1 == b2 == 0.  Then relu(dt*W1) == dt * max(W1, 0)
exactly, and the whole kernel-MLP collapses to a *constant* 32x32 matrix
V = (max(W1,0) @ W2).reshape(32,32).  Reassociating the contraction:

    out[b,i,:] = (sum_k dt_m[b,i,k] * features[b,i-k,:]) @ V = g[b,i,:] @ V

This is an exact algebraic identity for those inputs (verified by the guard
below at runtime; a general fallback handles anything else).

Per-core device program (core = 2*b + half, 1024 positions each):
  1. Feature windows fwin[p,t,q,:] = ft[128t+p+q, :] gathered straight from
     the padded f16 feature input in DRAM -- no staging, no dependencies, all
     8 tile gathers issue immediately across 4 queues.
  2. dt tiles [128 pos, 8 k] from shifted window loads of `times`, masked by
     a single host-precomputed band&length mask, cast to f16.
  3. g = sum_k dt*fwin via f16 broadcast-multiply + X-axis reduce on the DVE.
  4. Tail on the (otherwise idle) PE: transpose g via identity matmul
     ([128,128] per 4 tiles), then per-tile gT @ V in bf16; ACT copies
     PSUM->SBUF; output DMAs spread over the Sync and GpSimd queues.
"""

import os

import numpy as np

_STAGE = int(os.environ.get("KSTAGE", "0"))

KS = 8          # band width (kernel size)
B = 4
L = 2048
C = 32          # in channels
OUT = 32        # out channels
HALF = 1024     # positions per core
PAD = 8         # halo rows in front of each shard
SEQ = HALF + PAD
NT = HALF // 128  # 8 position-tiles per core
N_CORES = 8

_CACHE = {}


def _build_program_v2():
    from contextlib import ExitStack

    import concourse.bacc as bacc
    import concourse.bass as bass
    from concourse import mybir

    f32 = mybir.dt.float32
    f16 = mybir.dt.float16
    bf16 = mybir.dt.bfloat16

    nc = bacc.Bacc(
        "TRN2", target_bir_lowering=False, debug=False, num_devices=N_CORES
    )

    tm = nc.dram_tensor("tm", [SEQ], f32, kind="ExternalInput").ap()
    mk = nc.dram_tensor("mk", [128, NT * KS], f32, kind="ExternalInput").ap()
    ft = nc.dram_tensor("ft", [SEQ, C], f16, kind="ExternalInput").ap()
    vm = nc.dram_tensor("vm", [128, OUT], bf16, kind="ExternalInput").ap()
    idm = nc.dram_tensor("idm", [128, 128], f32, kind="ExternalInput").ap()
    out = nc.dram_tensor("out", [HALF, OUT], f32, kind="ExternalOutput").ap()

    ta = nc.alloc_sbuf_tensor("ta", [128, NT, KS + 1], f32).ap()
    mk_sb = nc.alloc_sbuf_tensor("mk_sb", [128, NT, KS], f32).ap()
    dtr = nc.alloc_sbuf_tensor("dtr", [128, NT, KS], f32).ap()
    dth = nc.alloc_sbuf_tensor("dth", [128, NT, KS], f16).ap()
    fwin = nc.alloc_sbuf_tensor("fwin", [128, NT, KS, C], f16).ap()
    # product [p, t, q, c] fully contiguous; summed over q by tree adds
    pr = nc.alloc_sbuf_tensor("pr", [128, NT, KS, C], f16).ap()
    s1 = nc.alloc_sbuf_tensor("s1", [128, NT, KS // 2, C], f16).ap()
    s2 = nc.alloc_sbuf_tensor("s2", [128, NT, KS // 4, C], f16).ap()
    oh = nc.alloc_sbuf_tensor("oh", [128, NT, C], f32).ap()
    gtc = [nc.alloc_sbuf_tensor(f"gtc{i}", [64, 128], bf16).ap() for i in range(4)]
    osb = nc.alloc_sbuf_tensor("osb", [128, NT, OUT], f32).ap()
    id_sb = nc.alloc_sbuf_tensor("id_sb", [128, 128], f32).ap()
    vm_sb = nc.alloc_sbuf_tensor("vm_sb", [128, OUT], bf16).ap()
    scr = nc.alloc_sbuf_tensor("scr", [1, 1], f32).ap()

    # one full PSUM bank per buffer so PE writes and ACT reads of
    # back-to-back stages never touch the same bank
    psT = [nc.alloc_psum_tensor(f"psT{i}", [128, 512], f32).ap() for i in range(2)]
    po = [nc.alloc_psum_tensor(f"po{i}", [128, 512], f32).ap() for i in range(4)]

    with ExitStack() as _sctx:
        block = _sctx.enter_context(nc.Block(no_gpsimd_drain=True))
        _names = ["sIN", "sMK", "sGA", "sGAg", "sGB", "sGBg", "sID",
                  "sVM", "sVD", "sGD", "sOH", "sOHg", "sPE", "sCP", "sOS",
                  "sOUT", "sOUTg"]
        _sems = {n: _sctx.enter_context(nc.semaphore(n)) for n in _names}
        (sIN, sMK, sGA, sGAg, sGB, sGBg, sID, sVM, sVD, sGD, sOH, sOHg,
         sPE, sCP, sOS, sOUT, sOUTg) = (_sems[n] for n in _names)

        def gather(raw, t, sem):
            # fwin[p, t, q, :] = ft[128t + p + q, :]; rows overlap, each
            # partition reads 8 contiguous 32-ch rows (512B) from DRAM.
            raw.dma_start(
                fwin[:, t, :, :],
                bass.AP(tensor=ft.tensor, offset=128 * t * C,
                        ap=[[C, 128], [C, KS], [1, C]]),
            ).then_inc(sem, 16)

        def slot(t):
            # 4 distinct PSUM out banks, matmul dst at bank col 0
            return po[t % 4][:, 0:OUT]

        def out_dma(raw, t, sem):
            raw.wait_ge(sOS, t + 1)
            raw.dma_start(
                bass.AP(tensor=out.tensor, offset=t * 128 * OUT,
                        ap=[[OUT, 128], [1, OUT]]),
                osb[:, t, :],
            ).then_inc(sem, 16)

        @block.sync
        def _(sync):
            sync.dma_start(
                ta[:],
                bass.AP(tensor=tm.tensor, offset=0,
                        ap=[[1, 128], [128, NT], [1, KS + 1]]),
            ).then_inc(sIN, 16)
            gather(sync, 0, sGA)
            gather(sync, 5, sGB)
            for t in (0, 1, 2, 3, 6):
                out_dma(sync, t, sOUT)
            sync.wait_ge(sOUT, 96)
            sync.wait_ge(sOUTg, 32)

        @block.gpsimd
        def _(g):
            g.dma_start(mk_sb[:], mk[:]).then_inc(sMK, 16)
            gather(g, 1, sGAg)
            gather(g, 4, sGBg)
            g.dma_start(id_sb[:], idm[:]).then_inc(sID, 16)
            g.dma_start(vm_sb[:], vm[:]).then_inc(sVM, 16)
            for t in (4, 5):
                out_dma(g, t, sOUTg)

        @block.scalar
        def _(s):
            gather(s, 2, sGA)
            gather(s, 3, sGA)
            gather(s, 6, sGB)
            gather(s, 7, sGB)
            # dummy activate: pulls the ACT table load off the critical path
            # (first ACTIVATE triggers a ~1.3us table fetch); osb[0,0,0] is
            # rewritten in-order by the real copy below.
            s.wait_ge(sMK, 16)
            nc.scalar.copy(scr[:], mk_sb[0:1, 0, 0:1])
            if _STAGE == 1:
                # debug: bypass PE tail, copy oh straight out (wrong values)
                for t in range(8):
                    s.wait_ge(sOH, 1 if t < 4 else 2)
                    nc.scalar.copy(osb[:, t, :], oh[:, t, :]).then_inc(sOS, 1)
            elif _STAGE == 2:
                # debug: transposes only; copy psT chunks out (wrong values)
                for g in range(4):
                    s.wait_ge(sPE, g + 1)
                    nc.scalar.copy(
                        osb[0:64, 2 * g:2 * g + 2, :], psT[g % 2][0:64, 0:64]
                    ).then_inc(sOS, 2)
            else:
                # (gtc chunk ready at sPE, src bank) then osb copies per MM
                s.wait_ge(sPE, 1)
                nc.scalar.copy(gtc[0][:], psT[0][0:64, 0:128]).then_inc(sCP, 1)
                s.wait_ge(sPE, 2)
                nc.scalar.copy(gtc[1][:], psT[1][0:64, 0:128]).then_inc(sCP, 1)
                for t in range(4):
                    s.wait_ge(sPE, t + 3)
                    nc.scalar.copy(osb[:, t, :], slot(t)).then_inc(sOS, 1)
                s.wait_ge(sPE, 7)
                nc.scalar.copy(gtc[2][:], psT[0][0:64, 0:128]).then_inc(sCP, 1)
                s.wait_ge(sPE, 8)
                nc.scalar.copy(osb[:, 4, :], slot(4)).then_inc(sOS, 1)
                s.wait_ge(sPE, 9)
                nc.scalar.copy(osb[:, 5, :], slot(5)).then_inc(sOS, 1)
                s.wait_ge(sPE, 10)
                nc.scalar.copy(gtc[3][:], psT[1][0:64, 0:128]).then_inc(sCP, 1)
                s.wait_ge(sPE, 11)
                nc.scalar.copy(osb[:, 6, :], slot(6)).then_inc(sOS, 1)
                s.wait_ge(sPE, 12)
                nc.scalar.copy(osb[:, 7, :], slot(7)).then_inc(sOS, 1)
                out_dma(s, 7, sOUT)

        @block.tensor
        def _(te):
            if _STAGE == 1:
                return
            if _STAGE == 2:
                te.wait_ge(sID, 16)
                for g in range(4):
                    te.wait_ge(sOH, 1 if g < 2 else 2)
                    if g >= 2:
                        te.wait_ge(sOS, 2 * g)  # psT[g%2] drained
                    nc.tensor.transpose(
                        psT[g % 2][0:64, 0:128],
                        oh[:, 2 * g:2 * g + 2, :]
                        .rearrange("p a b -> p (a b)"),
                        id_sb[:],
                    ).then_inc(sPE, 1)
                return
            # transposes per 2 tiles ([64,128] chunks; operand base
            # partitions are restricted to {0, 32, 64}), then gT @ V matmuls
            def trans(pair, bank):
                nc.tensor.transpose(
                    psT[bank][0:64, 0:128],
                    oh[:, 2 * pair:2 * pair + 2, :]
                    .rearrange("p a b -> p (a b)"),
                    id_sb[:],
                ).then_inc(sPE, 1)

            def mm(t):
                te.wait_ge(sCP, t // 2 + 1)
                if t >= 4:
                    te.wait_ge(sOS, t - 3)  # slot(t-4) drained
                pb = 32 * (t % 2)
                nc.tensor.matmul(
                    slot(t), gtc[t // 2][pb:pb + 32, :],
                    vm_sb[pb:pb + 32, :], start=True, stop=True,
                ).then_inc(sPE, 1)

            te.wait_ge(sOH, 1)
            te.wait_ge(sID, 16)
            trans(0, 0)
            trans(1, 1)
            te.wait_ge(sVM, 16)
            for t in (0, 1, 2, 3):
                mm(t)
            te.wait_ge(sOH, 2)
            trans(2, 0)  # psT[0] drained by gtc0 copy (sCP>=1 via mm waits)
            mm(4)
            mm(5)
            te.wait_ge(sOH, 3)
            trans(3, 1)
            mm(6)
            mm(7)

        @block.vector
        def _(v):
            v.wait_ge(sIN, 16)
            v.wait_ge(sMK, 16)
            nc.vector.tensor_tensor(
                dtr[:],
                ta[:, :, KS:KS + 1].to_broadcast([128, NT, KS]),
                ta[:, :, 0:KS],
                mybir.AluOpType.subtract,
            ).then_inc(sVD, 1)
            v.wait_ge(sVD, 1)
            nc.vector.tensor_tensor(
                dth[:], dtr[:], mk_sb[:], mybir.AluOpType.mult
            ).then_inc(sVD, 1)
            v.wait_ge(sVD, 2)
            nvd = [2]
            def chained(ins):
                nvd[0] += 1
                ins.then_inc(sVD, 1)
                v.wait_ge(sVD, nvd[0])
            for gi, (t0, nt) in enumerate(((0, 4), (4, 2), (6, 2))):
                if gi == 0:
                    v.wait_ge(sGA, 48)
                    v.wait_ge(sGAg, 16)
                elif gi == 1:
                    v.wait_ge(sGB, 48)
                    v.wait_ge(sGBg, 16)
                sl = slice(t0, t0 + nt)
                chained(nc.vector.tensor_tensor(
                    pr[:, sl],
                    dth[:, sl, :, None].to_broadcast([128, nt, KS, C]),
                    fwin[:, sl],
                    mybir.AluOpType.mult,
                ))
                chained(nc.vector.tensor_tensor(
                    s1[:, sl], pr[:, sl, 0:4, :], pr[:, sl, 4:8, :],
                    mybir.AluOpType.add,
                ))
                chained(nc.vector.tensor_tensor(
                    s2[:, sl], s1[:, sl, 0:2, :], s1[:, sl, 2:4, :],
                    mybir.AluOpType.add,
                ))
                nc.vector.tensor_tensor(
                    oh[:, sl], s2[:, sl, 0, :], s2[:, sl, 1, :],
                    mybir.AluOpType.add,
                ).then_inc(sOH, 1)

    nc.compile()
    return nc


def _get_program():
    if "nc" not in _CACHE:
        _CACHE["nc"] = _build_program_v2()
    return _CACHE["nc"]


def _fast_path_ok(times, b1, b2):
    # The linearization relu(dt*W1 + b1) == dt*max(W1,0) is exact iff
    # b1 == 0 and dt >= 0 (times sorted); b2 == 0 removes the bias term.
    if np.any(b1 != 0.0) or np.any(b2 != 0.0):
        return False
    if np.any(np.diff(times, axis=1) < 0.0):
        return False
    return True


def _reference_fallback(times, features, lengths, W1, b1, W2, b2):
    # Straight numpy transcription of the reference (general inputs).
    Bn, Ln = times.shape
    offsets = np.arange(1, KS + 1)
    idx = np.arange(Ln)[:, None] - offsets[None, :]
    in_band = idx >= 0
    idx_c = np.clip(idx, 0, Ln - 1)
    t_j = times[:, idx_c]
    dt = times[:, :, None] - t_j
    pos_i = np.arange(Ln)[None, :, None]
    mask = (
        in_band[None]
        & (idx_c[None] < lengths[:, None, None])
        & (pos_i <= lengths[:, None, None] - 1)
    )
    dt = np.where(mask, dt, 0.0).astype(np.float32)
    hidden = np.maximum(dt[..., None] * W1[0] + b1, 0.0)
    kv = (hidden @ W2 + b2).reshape(Bn, Ln, KS, C, OUT)
    kv = np.where(mask[..., None, None], kv, 0.0)
    feat_g = features[:, idx_c]
    return np.einsum("blkc,blkco->blo", feat_g, kv).astype(np.float32)


def _build_in_maps(times, features, lengths, W1, W2):
    from ml_dtypes import bfloat16

    # Fold the (now linear) kernel-MLP into one 32x32 matrix.
    v = (np.maximum(W1[0], 0.0) @ W2).reshape(C, OUT).astype(np.float32)
    vm16 = np.ascontiguousarray(np.tile(v, (4, 1))).astype(bfloat16)
    ident = np.eye(128, dtype=np.float32)

    p_loc = np.arange(128)[:, None, None] + 128 * np.arange(NT)[None, :, None]
    k = KS - np.arange(KS)[None, None, :]

    in_maps = []
    for core in range(N_CORES):
        b, half = core // 2, core % 2
        start = half * HALF
        ftp = np.zeros((SEQ, C), np.float16)
        tmv = np.empty((SEQ,), np.float32)
        lo = start - PAD
        if lo < 0:
            ftp[PAD:] = features[b, 0:start + HALF]
            tmv[:PAD] = times[b, 0]
            tmv[PAD:] = times[b, 0:start + HALF]
        else:
            ftp[:] = features[b, lo:start + HALF]
            tmv[:] = times[b, lo:start + HALF]
        # mask[p, t, q] = 1 iff global pos i = start+128t+p has i >= 8-q
        # (band: j = i-k >= 0, k = 8-q) and local pos < lengths[b]-start.
        band = (p_loc + start) >= k
        lenm = p_loc < (int(lengths[b]) - start)
        mkv = np.ascontiguousarray(
            (band & lenm).astype(np.float32).reshape(128, NT * KS)
        )
        in_maps.append({"tm": tmv, "mk": mkv, "ft": ftp, "vm": vm16,
                        "idm": ident})
    return in_maps


def kernel(times, features, lengths, W1, b1, W2, b2):
    times = np.asarray(times, dtype=np.float32)
    features = np.asarray(features, dtype=np.float32)
    lengths = np.asarray(lengths)
    W1 = np.asarray(W1, dtype=np.float32)
    b1 = np.asarray(b1, dtype=np.float32)
    W2 = np.asarray(W2, dtype=np.float32)
    b2 = np.asarray(b2, dtype=np.float32)

    if not _fast_path_ok(times, b1, b2):
        return _reference_fallback(times, features, lengths, W1, b1, W2, b2)

    from concourse.bass_utils import run_bass_kernel_spmd

    nc = _get_program()
    in_maps = _build_in_maps(times, features, lengths, W1, W2)
    res = run_bass_kernel_spmd(nc, in_maps, core_ids=list(range(N_CORES)))

    out = np.empty((B, L, OUT), np.float32)
    for core in range(N_CORES):
        b, half = core // 2, core % 2
        out[b, half * HALF : (half + 1) * HALF, :] = res.results[core]["out"]
    return out


# revision 17
# speedup vs baseline: 1.0661x; 1.0239x over previous
"""Trainium2 kernel for nn_ContConv1dDense (banded continuous conv with
kernel-MLP), data-parallel over (batch, sequence-half) on 8 NeuronCores.

Math: the reference computes, per (b, i, k in 1..8):
    dt      = (times[b,i] - times[b,i-k]) masked to the band & valid length
    hidden  = relu(dt * W1 + b1)                       # (128,)
    kv      = (hidden @ W2 + b2).reshape(32, 32)       # masked
    out[b,i,:] += features[b,i-k,:] @ kv

For this operator's input family, `times` is sorted along the sequence axis
(so dt >= 0) and b1 == b2 == 0.  Then relu(dt*W1) == dt * max(W1, 0)
exactly, and the whole kernel-MLP collapses to a *constant* 32x32 matrix
V = (max(W1,0) @ W2).reshape(32,32).  Reassociating the contraction:

    out[b,i,:] = (sum_k dt_m[b,i,k] * features[b,i-k,:]) @ V = g[b,i,:] @ V

This is an exact algebraic identity for those inputs (verified by the guard
below at runtime; a general fallback handles anything else).

Per-core device program (core = 2*b + half, 1024 positions each):
  1. Feature windows fwin[p,t,q,:] = ft[128t+p+q, :] gathered straight from
     the padded f16 feature input in DRAM -- no staging, no dependencies, all
     8 tile gathers issue immediately across 4 queues.
  2. dt tiles [128 pos, 8 k] from shifted window loads of `times`, masked by
     a single host-precomputed band&length mask, cast to f16.
  3. g = sum_k dt*fwin via f16 broadcast-multiply + X-axis reduce on the DVE.
  4. Tail on the (otherwise idle) PE: transpose g via identity matmul
     ([128,128] per 4 tiles), then per-tile gT @ V in bf16; ACT copies
     PSUM->SBUF; output DMAs spread over the Sync and GpSimd queues.
"""

import os

import numpy as np

_STAGE = int(os.environ.get("KSTAGE", "0"))

KS = 8          # band width (kernel size)
B = 4
L = 2048
C = 32          # in channels
OUT = 32        # out channels
HALF = 1024     # positions per core
PAD = 8         # halo rows in front of each shard
SEQ = HALF + PAD
NT = HALF // 128  # 8 position-tiles per core
N_CORES = 8

_CACHE = {}


def _build_program_v2():
    from contextlib import ExitStack

    import concourse.bacc as bacc
    import concourse.bass as bass
    from concourse import mybir

    f32 = mybir.dt.float32
    f16 = mybir.dt.float16
    bf16 = mybir.dt.bfloat16

    nc = bacc.Bacc(
        "TRN2", target_bir_lowering=False, debug=False, num_devices=N_CORES
    )

    tm = nc.dram_tensor("tm", [SEQ], f32, kind="ExternalInput").ap()
    mk = nc.dram_tensor("mk", [128, NT * KS], f32, kind="ExternalInput").ap()
    ft = nc.dram_tensor("ft", [SEQ, C], f16, kind="ExternalInput").ap()
    vm = nc.dram_tensor("vm", [128, OUT], bf16, kind="ExternalInput").ap()
    idm = nc.dram_tensor("idm", [128, 128], f32, kind="ExternalInput").ap()
    out = nc.dram_tensor("out", [HALF, OUT], f32, kind="ExternalOutput").ap()

    ta = nc.alloc_sbuf_tensor("ta", [128, NT, KS + 1], f32).ap()
    mk_sb = nc.alloc_sbuf_tensor("mk_sb", [128, NT, KS], f32).ap()
    dtr = nc.alloc_sbuf_tensor("dtr", [128, NT, KS], f32).ap()
    dth = nc.alloc_sbuf_tensor("dth", [128, NT, KS], f16).ap()
    fwin = nc.alloc_sbuf_tensor("fwin", [128, NT, KS, C], f16).ap()
    # product [p, t, q, c] fully contiguous; summed over q by tree adds
    pr = nc.alloc_sbuf_tensor("pr", [128, NT, KS, C], f16).ap()
    s1 = nc.alloc_sbuf_tensor("s1", [128, NT, KS // 2, C], f16).ap()
    s2 = nc.alloc_sbuf_tensor("s2", [128, NT, KS // 4, C], f16).ap()
    oh = nc.alloc_sbuf_tensor("oh", [128, NT, C], f32).ap()
    gtc = [nc.alloc_sbuf_tensor(f"gtc{i}", [64, 128], bf16).ap() for i in range(4)]
    osb = nc.alloc_sbuf_tensor("osb", [128, NT, OUT], f32).ap()
    id_sb = nc.alloc_sbuf_tensor("id_sb", [128, 128], f32).ap()
    vm_sb = nc.alloc_sbuf_tensor("vm_sb", [128, OUT], bf16).ap()
    scr = nc.alloc_sbuf_tensor("scr", [1, 1], f32).ap()

    # one full PSUM bank per buffer so PE writes and ACT reads of
    # back-to-back stages never touch the same bank
    psT = [nc.alloc_psum_tensor(f"psT{i}", [128, 512], f32).ap() for i in range(2)]
    po = [nc.alloc_psum_tensor(f"po{i}", [128, 512], f32).ap() for i in range(4)]

    with ExitStack() as _sctx:
        block = _sctx.enter_context(nc.Block(no_gpsimd_drain=True))
        _names = ["sIN", "sMK", "sGA", "sGAg", "sGB", "sGBg", "sID",
                  "sVM", "sVD", "sGD", "sOH", "sOHg", "sPE", "sCP", "sOS",
                  "sOUT", "sOUTg"]
        _sems = {n: _sctx.enter_context(nc.semaphore(n)) for n in _names}
        (sIN, sMK, sGA, sGAg, sGB, sGBg, sID, sVM, sVD, sGD, sOH, sOHg,
         sPE, sCP, sOS, sOUT, sOUTg) = (_sems[n] for n in _names)

        def gather(raw, t, sem):
            # fwin[p, t, q, :] = ft[128t + p + q, :]; rows overlap, each
            # partition reads 8 contiguous 32-ch rows (512B) from DRAM.
            raw.dma_start(
                fwin[:, t, :, :],
                bass.AP(tensor=ft.tensor, offset=128 * t * C,
                        ap=[[C, 128], [C, KS], [1, C]]),
            ).then_inc(sem, 16)

        def slot(t):
            # 4 distinct PSUM out banks, matmul dst at bank col 0
            return po[t % 4][:, 0:OUT]

        def out_dma(raw, t, sem):
            raw.wait_ge(sOS, t + 1)
            raw.dma_start(
                bass.AP(tensor=out.tensor, offset=t * 128 * OUT,
                        ap=[[OUT, 128], [1, OUT]]),
                osb[:, t, :],
            ).then_inc(sem, 16)

        @block.sync
        def _(sync):
            sync.dma_start(
                ta[:],
                bass.AP(tensor=tm.tensor, offset=0,
                        ap=[[1, 128], [128, NT], [1, KS + 1]]),
            ).then_inc(sIN, 16)
            gather(sync, 0, sGA)
            gather(sync, 5, sGB)
            for t in (0, 1, 2, 3, 6):
                out_dma(sync, t, sOUT)
            sync.wait_ge(sOUT, 96)
            sync.wait_ge(sOUTg, 32)

        @block.gpsimd
        def _(g):
            g.dma_start(mk_sb[:], mk[:]).then_inc(sMK, 16)
            gather(g, 1, sGAg)
            gather(g, 4, sGBg)
            g.dma_start(id_sb[:], idm[:]).then_inc(sID, 16)
            g.dma_start(vm_sb[:], vm[:]).then_inc(sVM, 16)
            for t in (4, 5):
                out_dma(g, t, sOUTg)

        @block.scalar
        def _(s):
            gather(s, 2, sGA)
            gather(s, 3, sGA)
            gather(s, 6, sGB)
            gather(s, 7, sGB)
            # dummy activate: pulls the ACT table load off the critical path
            # (first ACTIVATE triggers a ~1.3us table fetch); osb[0,0,0] is
            # rewritten in-order by the real copy below.
            s.wait_ge(sMK, 16)
            nc.scalar.copy(scr[:], mk_sb[0:1, 0, 0:1])
            if _STAGE == 1:
                # debug: bypass PE tail, copy oh straight out (wrong values)
                for t in range(8):
                    s.wait_ge(sOH, 1 if t < 4 else 2)
                    nc.scalar.copy(osb[:, t, :], oh[:, t, :]).then_inc(sOS, 1)
            elif _STAGE == 2:
                # debug: transposes only; copy psT chunks out (wrong values)
                for g in range(4):
                    s.wait_ge(sPE, g + 1)
                    nc.scalar.copy(
                        osb[0:64, 2 * g:2 * g + 2, :], psT[g % 2][0:64, 0:64]
                    ).then_inc(sOS, 2)
            else:
                # (gtc chunk ready at sPE, src bank) then osb copies per MM
                s.wait_ge(sPE, 1)
                nc.scalar.copy(gtc[0][:], psT[0][0:64, 0:128]).then_inc(sCP, 1)
                s.wait_ge(sPE, 2)
                nc.scalar.copy(gtc[1][:], psT[1][0:64, 0:128]).then_inc(sCP, 1)
                for t in range(4):
                    s.wait_ge(sPE, t + 3)
                    nc.scalar.copy(osb[:, t, :], slot(t)).then_inc(sOS, 1)
                s.wait_ge(sPE, 7)
                nc.scalar.copy(gtc[2][:], psT[0][0:64, 0:128]).then_inc(sCP, 1)
                s.wait_ge(sPE, 8)
                nc.scalar.copy(gtc[3][:], psT[1][0:64, 0:128]).then_inc(sCP, 1)
                for t in range(4, 8):
                    s.wait_ge(sPE, t + 5)
                    nc.scalar.copy(osb[:, t, :], slot(t)).then_inc(sOS, 1)
                out_dma(s, 7, sOUT)

        @block.tensor
        def _(te):
            if _STAGE == 1:
                return
            if _STAGE == 2:
                te.wait_ge(sID, 16)
                for g in range(4):
                    te.wait_ge(sOH, 1 if g < 2 else 2)
                    if g >= 2:
                        te.wait_ge(sOS, 2 * g)  # psT[g%2] drained
                    nc.tensor.transpose(
                        psT[g % 2][0:64, 0:128],
                        oh[:, 2 * g:2 * g + 2, :]
                        .rearrange("p a b -> p (a b)"),
                        id_sb[:],
                    ).then_inc(sPE, 1)
                return
            # transposes per 2 tiles ([64,128] chunks; operand base
            # partitions are restricted to {0, 32, 64}), then gT @ V matmuls
            def trans(pair, bank):
                nc.tensor.transpose(
                    psT[bank][0:64, 0:128],
                    oh[:, 2 * pair:2 * pair + 2, :]
                    .rearrange("p a b -> p (a b)"),
                    id_sb[:],
                ).then_inc(sPE, 1)

            def mm(t):
                te.wait_ge(sCP, t // 2 + 1)
                if t >= 4:
                    te.wait_ge(sOS, t - 3)  # slot(t-4) drained
                pb = 32 * (t % 2)
                nc.tensor.matmul(
                    slot(t), gtc[t // 2][pb:pb + 32, :],
                    vm_sb[pb:pb + 32, :], start=True, stop=True,
                ).then_inc(sPE, 1)

            te.wait_ge(sOH, 1)
            te.wait_ge(sID, 16)
            trans(0, 0)
            trans(1, 1)
            te.wait_ge(sVM, 16)
            for t in (0, 1, 2, 3):
                mm(t)
            te.wait_ge(sOH, 2)
            trans(2, 0)  # psT[0] drained by gtc0 copy (sCP>=1 via mm waits)
            trans(3, 1)
            for t in (4, 5, 6, 7):
                mm(t)

        @block.vector
        def _(v):
            v.wait_ge(sIN, 16)
            v.wait_ge(sMK, 16)
            nc.vector.tensor_tensor(
                dtr[:],
                ta[:, :, KS:KS + 1].to_broadcast([128, NT, KS]),
                ta[:, :, 0:KS],
                mybir.AluOpType.subtract,
            ).then_inc(sVD, 1)
            v.wait_ge(sVD, 1)
            nc.vector.tensor_tensor(
                dth[:], dtr[:], mk_sb[:], mybir.AluOpType.mult
            ).then_inc(sVD, 1)
            v.wait_ge(sVD, 2)
            nvd = [2]
            def chained(ins):
                nvd[0] += 1
                ins.then_inc(sVD, 1)
                v.wait_ge(sVD, nvd[0])
            for gi, (t0, nt) in enumerate(((0, 4), (4, 4)),):
                if gi == 0:
                    v.wait_ge(sGA, 48)
                    v.wait_ge(sGAg, 16)
                elif gi == 1:
                    v.wait_ge(sGB, 48)
                    v.wait_ge(sGBg, 16)
                sl = slice(t0, t0 + nt)
                chained(nc.vector.tensor_tensor(
                    pr[:, sl],
                    dth[:, sl, :, None].to_broadcast([128, nt, KS, C]),
                    fwin[:, sl],
                    mybir.AluOpType.mult,
                ))
                chained(nc.vector.tensor_tensor(
                    s1[:, sl], pr[:, sl, 0:4, :], pr[:, sl, 4:8, :],
                    mybir.AluOpType.add,
                ))
                chained(nc.vector.tensor_tensor(
                    s2[:, sl], s1[:, sl, 0:2, :], s1[:, sl, 2:4, :],
                    mybir.AluOpType.add,
                ))
                nc.vector.tensor_tensor(
                    oh[:, sl], s2[:, sl, 0, :], s2[:, sl, 1, :],
                    mybir.AluOpType.add,
                ).then_inc(sOH, 1)

    nc.compile()
    return nc


def _get_program():
    if "nc" not in _CACHE:
        _CACHE["nc"] = _build_program_v2()
    return _CACHE["nc"]


def _fast_path_ok(times, b1, b2):
    # The linearization relu(dt*W1 + b1) == dt*max(W1,0) is exact iff
    # b1 == 0 and dt >= 0 (times sorted); b2 == 0 removes the bias term.
    if np.any(b1 != 0.0) or np.any(b2 != 0.0):
        return False
    if np.any(np.diff(times, axis=1) < 0.0):
        return False
    return True


def _reference_fallback(times, features, lengths, W1, b1, W2, b2):
    # Straight numpy transcription of the reference (general inputs).
    Bn, Ln = times.shape
    offsets = np.arange(1, KS + 1)
    idx = np.arange(Ln)[:, None] - offsets[None, :]
    in_band = idx >= 0
    idx_c = np.clip(idx, 0, Ln - 1)
    t_j = times[:, idx_c]
    dt = times[:, :, None] - t_j
    pos_i = np.arange(Ln)[None, :, None]
    mask = (
        in_band[None]
        & (idx_c[None] < lengths[:, None, None])
        & (pos_i <= lengths[:, None, None] - 1)
    )
    dt = np.where(mask, dt, 0.0).astype(np.float32)
    hidden = np.maximum(dt[..., None] * W1[0] + b1, 0.0)
    kv = (hidden @ W2 + b2).reshape(Bn, Ln, KS, C, OUT)
    kv = np.where(mask[..., None, None], kv, 0.0)
    feat_g = features[:, idx_c]
    return np.einsum("blkc,blkco->blo", feat_g, kv).astype(np.float32)


def _build_in_maps(times, features, lengths, W1, W2):
    from ml_dtypes import bfloat16

    # Fold the (now linear) kernel-MLP into one 32x32 matrix.
    v = (np.maximum(W1[0], 0.0) @ W2).reshape(C, OUT).astype(np.float32)
    vm16 = np.ascontiguousarray(np.tile(v, (4, 1))).astype(bfloat16)
    ident = np.eye(128, dtype=np.float32)

    p_loc = np.arange(128)[:, None, None] + 128 * np.arange(NT)[None, :, None]
    k = KS - np.arange(KS)[None, None, :]

    in_maps = []
    for core in range(N_CORES):
        b, half = core // 2, core % 2
        start = half * HALF
        ftp = np.zeros((SEQ, C), np.float16)
        tmv = np.empty((SEQ,), np.float32)
        lo = start - PAD
        if lo < 0:
            ftp[PAD:] = features[b, 0:start + HALF]
            tmv[:PAD] = times[b, 0]
            tmv[PAD:] = times[b, 0:start + HALF]
        else:
            ftp[:] = features[b, lo:start + HALF]
            tmv[:] = times[b, lo:start + HALF]
        # mask[p, t, q] = 1 iff global pos i = start+128t+p has i >= 8-q
        # (band: j = i-k >= 0, k = 8-q) and local pos < lengths[b]-start.
        band = (p_loc + start) >= k
        lenm = p_loc < (int(lengths[b]) - start)
        mkv = np.ascontiguousarray(
            (band & lenm).astype(np.float32).reshape(128, NT * KS)
        )
        in_maps.append({"tm": tmv, "mk": mkv, "ft": ftp, "vm": vm16,
                        "idm": ident})
    return in_maps


def kernel(times, features, lengths, W1, b1, W2, b2):
    times = np.asarray(times, dtype=np.float32)
    features = np.asarray(features, dtype=np.float32)
    lengths = np.asarray(lengths)
    W1 = np.asarray(W1, dtype=np.float32)
    b1 = np.asarray(b1, dtype=np.float32)
    W2 = np.asarray(W2, dtype=np.float32)
    b2 = np.asarray(b2, dtype=np.float32)

    if not _fast_path_ok(times, b1, b2):
        return _reference_fallback(times, features, lengths, W1, b1, W2, b2)

    from concourse.bass_utils import run_bass_kernel_spmd

    nc = _get_program()
    in_maps = _build_in_maps(times, features, lengths, W1, W2)
    res = run_bass_kernel_spmd(nc, in_maps, core_ids=list(range(N_CORES)))

    out = np.empty((B, L, OUT), np.float32)
    for core in range(N_CORES):
        b, half = core // 2, core % 2
        out[b, half * HALF : (half + 1) * HALF, :] = res.results[core]["out"]
    return out
